# revision 23
# baseline (speedup 1.0000x reference)
"""EquiConv (DeepH-E3) Trainium2 kernel — 8-core data-parallel over edges.

Strategy (channel-major, bf16, 13 matmul slots per 1024-edge supertile):
  - Host folds per-channel weights/constants into matmul weights, casts
    everything to bf16, shards edges across 8 cores, pads to 25*1024 and
    transposes edge tensors to channel-major [C, E].  Per-edge scalars
    (s, v0, v1, v2 from fea_in2) are shipped pre-replicated across
    partitions in bf16; x1s / x1v01 / fw / rep_s / rep_v01 are packed
    into ONE interleaved "bigin" DRAM tensor so each supertile needs
    only 3 load DMAs + 2 store DMAs.
  - Matmul packing: gate and the p2 output are duplicated across both
    PSUM halves via widened weight matrices (idle M side), p3's wc
    matmuls for components 0/1 are merged into one block-diagonal
    matmul, and the vec-path adds are folded into PSUM accumulation by
    seeding the bank with the DVE product (matmul start=False).
  - Elementwise split: GpSimd(Pool) takes the three big SBUF-only
    prescales, ACT takes activations + the wwa/wwb PSUM evacuations,
    DVE takes the PSUM-touching products and bf16 finals (2x mode).
  - Host transposes the bf16 channel-major output back to [E, 320] f32.

Self-contained: hardcodes shapes from the problem spec; no file reads.
"""
import os
import sys

import numpy as np

# ---------------------------------------------------------------- constants
E_FULL = 200000
N_CORES = 8
E_CORE = E_FULL // N_CORES      # 25000
NT = 1024                       # edges per supertile
T_TILES = 25                    # supertiles per core
E_PAD = NT * T_TILES            # 25600
NH = 512                        # psum half
MUL_S = 128
MUL_V = 64

INV_S = 1.0 / np.sqrt(MUL_S)
INV_V = 1.0 / np.sqrt(MUL_V)
SQ2 = 1.0 / np.sqrt(2.0)
SQ3 = 1.0 / np.sqrt(3.0)

BIGIN_W = 5 * NT                # x1s | x1v01 | fw | rep_s | rep_v01
BIGOUT_W = 2 * NT               # out_s | out01

_REPO_CANDIDATES = (
    "/opt/trn_rl_repo",
    "/root/.axon_site/_ro/trn_rl_repo",
)


def _ensure_repo_on_path():
    try:
        import concourse.bass  # noqa: F401
        return
    except ImportError:
        pass
    for p in _REPO_CANDIDATES:
        if os.path.isdir(p) and p not in sys.path:
            sys.path.insert(0, p)
    import concourse.bass  # noqa: F401


_CACHE = {}
_SIM_SAFE_ACT = False   # replace Silu->Sigmoid so CoreSim can run


def _build_nc():
    """Build + compile the per-core Bass program (cached)."""
    if "nc" in _CACHE:
        return _CACHE["nc"]
    _ensure_repo_on_path()
    import concourse.mybir as mybir
    import concourse.tile as tile
    from concourse import bacc

    F32 = mybir.dt.float32
    BF16 = mybir.dt.bfloat16
    MULT = mybir.AluOpType.mult
    ADD = mybir.AluOpType.add

    class AF:
        Silu = (mybir.ActivationFunctionType.Sigmoid if _SIM_SAFE_ACT
                else mybir.ActivationFunctionType.Silu)
        Tanh = mybir.ActivationFunctionType.Tanh
        Identity = mybir.ActivationFunctionType.Identity

    nc = bacc.Bacc(trn_type="TRN2", target_bir_lowering=False, debug=False,
                   num_devices=N_CORES)

    # DRAM inputs (per-core shard) -----------------------------------------
    d_bigin = nc.dram_tensor("bigin", [128, T_TILES * BIGIN_W], BF16,
                             kind="ExternalInput")
    d_x1v2d = nc.dram_tensor("x1v2d", [128, E_PAD], BF16,
                             kind="ExternalInput")
    d_repsv2 = nc.dram_tensor("repsv2", [128, E_PAD], BF16,
                              kind="ExternalInput")
    # folded weights ([K, M] layouts, ready as lhsT)
    d_wa0 = nc.dram_tensor("wa0", [128, 128], BF16, kind="ExternalInput")
    d_wa1d = nc.dram_tensor("wa1d", [128, 128], BF16, kind="ExternalInput")
    d_wb4s = nc.dram_tensor("wb4s", [128, 128], BF16, kind="ExternalInput")
    d_wb5sd = nc.dram_tensor("wb5sd", [128, 128], BF16, kind="ExternalInput")
    d_wb4b = nc.dram_tensor("wb4b", [64, 128], BF16, kind="ExternalInput")
    d_wb5bd = nc.dram_tensor("wb5bd", [64, 128], BF16, kind="ExternalInput")
    d_wp2d = nc.dram_tensor("wp2d", [128, 128], BF16, kind="ExternalInput")
    d_wcd = nc.dram_tensor("wcd", [128, 128], BF16, kind="ExternalInput")
    d_wc = nc.dram_tensor("wc", [64, 64], BF16, kind="ExternalInput")
    d_fc0 = nc.dram_tensor("fc0", [128, 64], BF16, kind="ExternalInput")
    d_fc1 = nc.dram_tensor("fc1", [64, 64], BF16, kind="ExternalInput")
    d_fc2a = nc.dram_tensor("fc2a", [64, 128], BF16, kind="ExternalInput")
    d_fc2bd = nc.dram_tensor("fc2bd", [64, 128], BF16, kind="ExternalInput")
    d_b0 = nc.dram_tensor("b0c", [64, 1], F32, kind="ExternalInput")
    d_b1 = nc.dram_tensor("b1c", [64, 1], F32, kind="ExternalInput")
    d_b2a = nc.dram_tensor("b2a", [128, 1], F32, kind="ExternalInput")
    d_b2bd = nc.dram_tensor("b2bd", [128, 1], F32, kind="ExternalInput")

    d_bigout = nc.dram_tensor("bigout", [128, T_TILES * BIGOUT_W], BF16,
                              kind="ExternalOutput")
    d_out2 = nc.dram_tensor("out2", [64, E_PAD], BF16, kind="ExternalOutput")

    with tile.TileContext(nc) as tc:
        with tc.tile_pool(name="const", bufs=1) as cp, \
             tc.tile_pool(name="io", bufs=3) as io, \
             tc.tile_pool(name="wk", bufs=3) as wk, \
             tc.tile_pool(name="ot", bufs=3) as ot, \
             tc.tile_pool(name="ps", bufs=1, space="PSUM") as ps:

            def const(d, shape, dtype=BF16, lo=0):
                t = cp.tile(shape, dtype, name=d.name + "_sb")
                if lo:
                    nc.sync.dma_start(t[lo:128, :], d.ap())
                else:
                    nc.sync.dma_start(t, d.ap())
                return t

            w_wa0 = const(d_wa0, [128, 128])
            w_wa1d = const(d_wa1d, [128, 128])
            w_wb4s = const(d_wb4s, [128, 128])
            w_wb5sd = const(d_wb5sd, [128, 128])
            w_wp2d = const(d_wp2d, [128, 128])
            w_wcd = const(d_wcd, [128, 128])
            w_fc0 = const(d_fc0, [128, 64])
            w_wc = const(d_wc, [64, 64])
            w_fc1 = const(d_fc1, [64, 64])
            # weights living at partitions 64-127 (rhs at partition offset 64)
            w_wb4b = const(d_wb4b, [128, 128], lo=64)
            w_wb5bd = const(d_wb5bd, [128, 128], lo=64)
            w_fc2a = const(d_fc2a, [128, 128], lo=64)
            w_fc2bd = const(d_fc2bd, [128, 128], lo=64)
            c_b0 = const(d_b0, [64, 1], F32)
            c_b1 = const(d_b1, [128, 1], F32, lo=64)
            c_b2a = const(d_b2a, [128, 1], F32)
            c_b2bd = const(d_b2bd, [128, 1], F32)

            # Warm up the seed-accumulated PSUM banks: T8/T9 accumulate
            # with start=False onto DVE-seeded banks; the very first
            # group on a virgin bank misbehaves unless the bank has
            # hosted a started accumulation group once.
            zwarm = cp.tile([128, NH], BF16, name="zwarm")
            nc.vector.memset(zwarm, 0)
            for tag in ("c01", "c2"):
                pw = ps.tile([128, NH], F32, tag=tag)
                nc.tensor.matmul(pw, w_wa0, zwarm, start=True, stop=True)

            # software-pipelined emission: the wwa/wwb matmuls, their
            # evacuations, and the DVE tails + stores for half X are all
            # drained at the start of half X+1, so the PE never waits on
            # the in-half silu chain and every engine queue keeps the
            # seed/tail hazard alternation (no in-order deadlocks).
            pend = {}

            def drain_pend():
                if not pend:
                    return
                c = dict(pend)
                pend.clear()
                hs = c["hs"]
                nc.tensor.matmul(c["wwa"], w_fc2a[64:128, :],
                                 c["h2s"][64:128, hs],
                                 start=True, stop=True,
                                 tile_position=(64, 0))              # M3
                nc.tensor.matmul(c["wwb2"], w_fc2bd[64:128, :],
                                 c["h2s"][64:128, hs],
                                 start=True, stop=True,
                                 tile_position=(64, 0))              # M4
                nc.scalar.activation(c["wwa_sb"][:, hs], c["wwa"],
                                     AF.Identity, bias=c_b2a)        # A6
                nc.scalar.activation(c["wwb_sb"][:, hs], c["wwb2"],
                                     AF.Identity, bias=c_b2bd,
                                     scale=0.5)                      # A5
                # sgw2 = (tanh(g/2)+1) * (0.5*wwb2+0.5*b2b) = sig(g)*w_b
                nc.vector.scalar_tensor_tensor(c["sgw2"][:, hs],
                                               c["tg"][:, hs], 1.0,
                                               c["wwb_sb"][:, hs],
                                               ADD, MULT)
                # out_s = silu(scal) * (wwa + b2a)
                nc.vector.tensor_tensor(
                    c["bigout"][:, c["h"] * NH:(c["h"] + 1) * NH],
                    c["sc_silu"][:, hs], c["wwa_sb"][:, hs], MULT)
                nc.vector.tensor_tensor(
                    c["bigout"][:, NT + c["h"] * NH:NT + (c["h"] + 1) * NH],
                    c["c01"], c["sgw2"][:, hs], MULT)
                nc.vector.tensor_tensor(
                    c["out2"][64:128, hs], c["c2"][64:128, :],
                    c["sgw2"][64:128, hs], MULT)
                if c["h"] == 1:
                    t = c["t"]
                    nc.sync.dma_start(
                        d_bigout.ap()[:, t * BIGOUT_W:(t + 1) * BIGOUT_W],
                        c["bigout"])
                    nc.sync.dma_start(d_out2.ap()[:, t * NT:(t + 1) * NT],
                                      c["out2"][64:128, :])

            for t in range(T_TILES):
                sl = slice(t * NT, (t + 1) * NT)

                # ---- loads -------------------------------------------
                bigin = io.tile([128, BIGIN_W], BF16)
                x1v2d = io.tile([128, NT], BF16)
                repsv2 = io.tile([128, NT], BF16)
                if t == 0:
                    # split the first load into blocks ordered by first
                    # use, so the PE isn't gated on one 1.3MB transfer
                    nc.sync.dma_start(x1v2d, d_x1v2d.ap()[:, sl])
                    nc.sync.dma_start(repsv2, d_repsv2.ap()[:, sl])
                    for k in (0, 2, 3, 1, 4):   # x1s, fw, rep_s, x1v01, ...
                        nc.sync.dma_start(
                            bigin[:, k * NT:(k + 1) * NT],
                            d_bigin.ap()[:, t * BIGIN_W + k * NT:
                                         t * BIGIN_W + (k + 1) * NT])
                else:
                    nc.sync.dma_start(
                        bigin, d_bigin.ap()[:, t * BIGIN_W:(t + 1) * BIGIN_W])
                    nc.sync.dma_start(x1v2d, d_x1v2d.ap()[:, sl])
                    nc.sync.dma_start(repsv2, d_repsv2.ap()[:, sl])

                x1s = bigin[:, 0 * NT:1 * NT]
                x1v01 = bigin[:, 1 * NT:2 * NT]
                fw = bigin[:, 2 * NT:3 * NT]
                rep_s = bigin[:, 3 * NT:4 * NT]
                rep_v01 = bigin[:, 4 * NT:5 * NT]

                # ---- prescales: Pool (sbuf bf16), PE-consumption order.
                # First supertiles go on the (then idle) DVE so the PE
                # isn't gated on the slow Pool engine at startup.
                eng1 = nc.vector if t == 0 else nc.gpsimd
                eng2 = nc.vector if t <= 1 else nc.gpsimd
                # xvps2 = [x1v2*s (lo) ; x1v2*v2 (hi)]
                xvps2 = wk.tile([128, NT], BF16)
                eng2.tensor_tensor(xvps2, x1v2d, repsv2, MULT)
                xs_s = wk.tile([128, NT], BF16)
                eng2.tensor_tensor(xs_s, x1s, rep_s, MULT)
                xv_p01 = wk.tile([128, NT], BF16)
                eng1.tensor_tensor(xv_p01, x1v01, rep_v01, MULT)
                xv_s01 = wk.tile([128, NT], BF16)
                eng1.tensor_tensor(xv_s01, x1v01, rep_s, MULT)

                # ---- SBUF result tiles -------------------------------
                h1s = wk.tile([64, NT], BF16)
                h2s = wk.tile([128, NT], BF16)
                sc_silu = wk.tile([128, NT], BF16)
                tg = wk.tile([128, NT], BF16)
                wwa_sb = wk.tile([128, NT], BF16)
                wwb_sb = wk.tile([128, NT], BF16)
                sgw2 = wk.tile([128, NT], BF16)
                bigout = ot.tile([128, BIGOUT_W], BF16)
                out2 = ot.tile([128, NT], BF16)

                for h in range(2):
                    hs = slice(h * NH, (h + 1) * NH)

                    a2 = ps.tile([128, NH], F32, tag="a2")
                    scal = ps.tile([128, NH], F32, tag="scal")
                    gate2 = ps.tile([128, NH], F32, tag="gate2")
                    mlp = ps.tile([128, NH], F32, tag="mlp")
                    c01 = ps.tile([128, NH], F32, tag="c01")
                    c2 = ps.tile([128, NH], F32, tag="c2")
                    wwa = ps.tile([128, NH], F32, tag="wwa")
                    wwb2 = ps.tile([128, NH], F32, tag="wwb2")

                    # ---- PE stream (readiness order) -----------------
                    nc.tensor.matmul(a2, w_wp2d, x1s[:, hs],
                                     start=True, stop=True)          # T7
                    nc.tensor.matmul(mlp[0:64, :], w_fc0, fw[:, hs],
                                     start=True, stop=True)          # M1
                    drain_pend()          # prev half: M3/M4, A6/A5, tails
                    nc.tensor.matmul(scal, w_wb4b[64:128, :],
                                     xvps2[64:128, hs],
                                     start=True, stop=False,
                                     tile_position=(64, 0))          # T3
                    nc.tensor.matmul(gate2, w_wb5bd[64:128, :],
                                     xvps2[64:128, hs],
                                     start=True, stop=False,
                                     tile_position=(64, 0))          # T6
                    nc.tensor.matmul(scal, w_wa0, xs_s[:, hs],
                                     start=False, stop=False)        # T1
                    nc.tensor.matmul(gate2, w_wa1d, xs_s[:, hs],
                                     start=False, stop=False)        # T4
                    nc.tensor.matmul(scal, w_wb4s, xv_p01[:, hs],
                                     start=False, stop=True)         # T2
                    nc.tensor.matmul(gate2, w_wb5sd, xv_p01[:, hs],
                                     start=False, stop=True)         # T5

                    # seed vec banks with the p2 postscale, then let the
                    # wc matmuls accumulate on top (start=False)
                    nc.vector.tensor_tensor(c01, a2, rep_v01[:, hs], MULT)
                    nc.vector.tensor_tensor(c2[64:128, :], a2[64:128, :],
                                            repsv2[64:128, hs], MULT)

                    # ---- radial MLP (wwa/wwb deferred to next half) --
                    nc.scalar.activation(h1s[:, hs], mlp[0:64, :], AF.Silu,
                                         bias=c_b0)                  # A1
                    nc.tensor.matmul(mlp[64:128, :], w_fc1, h1s[:, hs],
                                     start=True, stop=True,
                                     tile_position=(0, 64))          # M2
                    nc.tensor.matmul(c01, w_wcd, xv_s01[:, hs],
                                     start=False, stop=True,
                                     skip_group_check=True)          # T8
                    nc.tensor.matmul(c2[64:128, :], w_wc,
                                     xvps2[0:64, hs],
                                     start=False, stop=True,
                                     skip_group_check=True,
                                     tile_position=(0, 64))          # T9

                    nc.scalar.activation(sc_silu[:, hs], scal, AF.Silu)
                    nc.scalar.activation(tg[:, hs], gate2, AF.Tanh,
                                         scale=0.5)
                    nc.scalar.activation(h2s[64:128, hs], mlp[64:128, :],
                                         AF.Silu,
                                         bias=c_b1[64:128, :])       # A2
                    pend.update(t=t, h=h, hs=hs, h2s=h2s, wwa=wwa,
                                wwb2=wwb2, c01=c01, c2=c2,
                                sc_silu=sc_silu, tg=tg, wwa_sb=wwa_sb,
                                wwb_sb=wwb_sb, sgw2=sgw2, bigout=bigout,
                                out2=out2)

            drain_pend()

    nc.compile()
    _CACHE["nc"] = nc
    return nc


def _bf16(x):
    import ml_dtypes
    return np.asarray(x, dtype=np.float32).astype(ml_dtypes.bfloat16)


def _fold_weights(inp):
    """Fold per-channel weights + constants into matmul matrices (bf16)."""
    f = lambda k: np.asarray(inp[k], dtype=np.float32)
    w0f = f("w1_p0") * f("w2_p0")[None, :] * (INV_S * SQ2)
    w1f = f("w1_p1") * f("w2_p1")[None, :] * (INV_S * SQ2)
    w2f = f("w1_p2") * f("w2_p2")[None, :] * (INV_S * SQ2)
    w3f = f("w1_p3") * f("w2_p3")[None, :] * (INV_V * SQ2)
    w4f = f("w1_p4") * f("w2_p4")[None, :] * (INV_V * SQ3 * SQ2)
    w5f = f("w1_p5") * f("w2_p5")[None, :] * (INV_V * SQ3 * SQ2)
    fc2 = f("fc_w2")
    b2 = f("fc_b2")
    wcd = np.zeros((128, 128), np.float32)
    wcd[:64, :64] = w3f
    wcd[64:, 64:] = w3f
    c = np.ascontiguousarray
    return {
        "wa0": _bf16(w0f),
        "wa1d": _bf16(np.concatenate([w1f, w1f], axis=1)),
        "wb4s": _bf16(np.concatenate([w4f, w4f], axis=0)),
        "wb5sd": _bf16(np.tile(w5f, (2, 2))),
        "wb4b": _bf16(w4f),
        "wb5bd": _bf16(np.concatenate([w5f, w5f], axis=1)),
        "wp2d": _bf16(np.concatenate([w2f, w2f], axis=1)),
        "wcd": _bf16(wcd),
        "wc": _bf16(w3f),
        "fc0": _bf16(f("fc_w0")),
        "fc1": _bf16(f("fc_w1")),
        "fc2a": _bf16(fc2[:, :128]),
        "fc2bd": _bf16(np.concatenate([fc2[:, 128:], fc2[:, 128:]], axis=1)),
        "b0c": c(f("fc_b0")[:, None]),
        "b1c": c(f("fc_b1")[:, None]),
        "b2a": c(b2[:128, None]),
        "b2bd": c(0.5 * np.concatenate([b2[128:], b2[128:]])[:, None]),
    }


def _shard_inputs(inp):
    """Per-core bf16 channel-major shards (padded to E_PAD edges)."""
    import ml_dtypes
    BF = ml_dtypes.bfloat16
    fea_in1 = np.asarray(inp["fea_in1"], dtype=np.float32)
    fea_in2 = np.asarray(inp["fea_in2"], dtype=np.float32)
    fea_w = np.asarray(inp["fea_weight"], dtype=np.float32)
    shards = []
    for c in range(N_CORES):
        s = slice(c * E_CORE, (c + 1) * E_CORE)
        x1 = fea_in1[s]
        x2 = fea_in2[s]
        fwm = fea_w[s]

        x1s_t = np.zeros((128, E_PAD), BF)
        x1s_t[:, :E_CORE] = x1[:, :128].T
        x1v = x1[:, 128:].reshape(E_CORE, 64, 3)
        x1v01_t = np.zeros((128, E_PAD), BF)
        x1v01_t[:64, :E_CORE] = x1v[:, :, 0].T
        x1v01_t[64:, :E_CORE] = x1v[:, :, 1].T
        x1v2d_t = np.zeros((128, E_PAD), BF)
        x1v2d_t[:64, :E_CORE] = x1v[:, :, 2].T
        x1v2d_t[64:, :E_CORE] = x1v[:, :, 2].T
        fw_t = np.zeros((128, E_PAD), BF)
        fw_t[:, :E_CORE] = fwm.T
        x2p = np.zeros((E_PAD, 4), np.float32)
        x2p[:E_CORE] = x2

        rep_s = np.broadcast_to(
            x2p[:, 0].astype(BF)[None, :], (128, E_PAD))
        rep_v01 = np.empty((128, E_PAD), BF)
        rep_v01[:64] = x2p[:, 1].astype(BF)
        rep_v01[64:] = x2p[:, 2].astype(BF)
        repsv2 = np.empty((128, E_PAD), BF)
        repsv2[:64] = x2p[:, 0].astype(BF)
        repsv2[64:] = x2p[:, 3].astype(BF)

        # interleave into bigin: [128, T, 5, NT]
        big = np.stack([
            x1s_t.reshape(128, T_TILES, NT),
            x1v01_t.reshape(128, T_TILES, NT),
            fw_t.reshape(128, T_TILES, NT),
            np.ascontiguousarray(rep_s).reshape(128, T_TILES, NT),
            rep_v01.reshape(128, T_TILES, NT),
        ], axis=2)                                  # [128, T, 5, NT]
        shards.append({
            "bigin": np.ascontiguousarray(
                big.reshape(128, T_TILES * BIGIN_W)),
            "x1v2d": x1v2d_t,
            "repsv2": repsv2,
        })
    return shards


def run(inputs, trace=False, trace_kwargs=None):
    """Run the kernel; returns (output [E,320] f32, BassKernelResults)."""
    _ensure_repo_on_path()
    from concourse import bass_utils

    nc = _build_nc()
    weights = _fold_weights(inputs)
    shards = _shard_inputs(inputs)
    in_maps = [{**weights, **sh} for sh in shards]

    kwargs = {}
    if trace:
        _install_ntff_hook()
        kwargs.update(trace=True, **(trace_kwargs or {}))
    res = bass_utils.run_bass_kernel_spmd(
        nc, in_maps, core_ids=list(range(N_CORES)), **kwargs)

    out = np.empty((E_FULL, 320), np.float32)
    for c in range(N_CORES):
        bo = np.asarray(res.results[c]["bigout"], dtype=np.float32)
        o2 = np.asarray(res.results[c]["out2"], dtype=np.float32)
        bo = bo.reshape(128, T_TILES, 2, NT)
        out_s = bo[:, :, 0, :].reshape(128, E_PAD)[:, :E_CORE]
        out01 = bo[:, :, 1, :].reshape(128, E_PAD)[:, :E_CORE]
        s = slice(c * E_CORE, (c + 1) * E_CORE)
        out[s, :128] = out_s.T
        # vec layout: out[e, 128 + u*3 + i]
        vec = np.empty((E_CORE, 64, 3), np.float32)
        vec[:, :, 0] = out01[:64].T
        vec[:, :, 1] = out01[64:].T
        vec[:, :, 2] = o2[:, :E_CORE].T
        out[s, 128:] = vec.reshape(E_CORE, 192)
    return out, res


def _install_ntff_hook():
    """Shim the missing antenv.axon_hooks so trace=True works under axon."""
    import types
    import antenv
    from concourse import bass_utils
    if "antenv.axon_hooks" in sys.modules:
        return
    mod = types.ModuleType("antenv.axon_hooks")
    _h = [None]
    mod.set_axon_ntff_profile_hook = lambda h: _h.__setitem__(0, h)
    mod.get_axon_ntff_profile_hook = lambda: _h[0]
    sys.modules["antenv.axon_hooks"] = mod
    antenv.axon_hooks = mod
    from trn_agent_boot.trn_boot import _ntff_profile_via_ctypes
    mod.set_axon_ntff_profile_hook(
        _ntff_profile_via_ctypes("/opt/axon/libaxon_pjrt.so"))
    bass_utils.upload_artifacts = lambda tmpdir: tmpdir


def kernel(**inputs) -> np.ndarray:
    out, _ = run(inputs, trace=False)
    return out


# revision 28
# speedup vs baseline: 1.0396x; 1.0396x over previous
"""EquiConv (DeepH-E3) Trainium2 kernel — 8-core data-parallel over edges.

Strategy (channel-major, bf16, 13 matmul slots per 1024-edge supertile):
  - Host folds per-channel weights/constants into matmul weights, casts
    everything to bf16, shards edges across 8 cores, pads to 25*1024 and
    transposes edge tensors to channel-major [C, E].  Per-edge scalars
    (s, v0, v1, v2 from fea_in2) are shipped pre-replicated across
    partitions in bf16; x1s / x1v01 / fw / rep_s / rep_v01 are packed
    into ONE interleaved "bigin" DRAM tensor so each supertile needs
    only 3 load DMAs + 2 store DMAs.
  - Matmul packing: gate and the p2 output are duplicated across both
    PSUM halves via widened weight matrices (idle M side), p3's wc
    matmuls for components 0/1 are merged into one block-diagonal
    matmul, and the vec-path adds are folded into PSUM accumulation by
    seeding the bank with the DVE product (matmul start=False).
  - Elementwise split: GpSimd(Pool) takes the three big SBUF-only
    prescales, ACT takes activations + the wwa/wwb PSUM evacuations,
    DVE takes the PSUM-touching products and bf16 finals (2x mode).
  - Host transposes the bf16 channel-major output back to [E, 320] f32.

Self-contained: hardcodes shapes from the problem spec; no file reads.
"""
import os
import sys

import numpy as np

# ---------------------------------------------------------------- constants
E_FULL = 200000
N_CORES = 8
E_CORE = E_FULL // N_CORES      # 25000
NT = 1024                       # edges per supertile
T_TILES = 25                    # supertiles per core
E_PAD = NT * T_TILES            # 25600
NH = 512                        # psum half
MUL_S = 128
MUL_V = 64

INV_S = 1.0 / np.sqrt(MUL_S)
INV_V = 1.0 / np.sqrt(MUL_V)
SQ2 = 1.0 / np.sqrt(2.0)
SQ3 = 1.0 / np.sqrt(3.0)

BIGIN_W = 5 * NT                # x1s | x1v01 | fw | rep_s | rep_v01
BIGOUT_W = 2 * NT               # out_s | out01

_REPO_CANDIDATES = (
    "/opt/trn_rl_repo",
    "/root/.axon_site/_ro/trn_rl_repo",
)


def _ensure_repo_on_path():
    try:
        import concourse.bass  # noqa: F401
        return
    except ImportError:
        pass
    for p in _REPO_CANDIDATES:
        if os.path.isdir(p) and p not in sys.path:
            sys.path.insert(0, p)
    import concourse.bass  # noqa: F401


_CACHE = {}
_SIM_SAFE_ACT = False   # replace Silu->Sigmoid so CoreSim can run


def _build_nc():
    """Build + compile the per-core Bass program (cached)."""
    if "nc" in _CACHE:
        return _CACHE["nc"]
    _ensure_repo_on_path()
    import concourse.mybir as mybir
    import concourse.tile as tile
    from concourse import bacc

    F32 = mybir.dt.float32
    BF16 = mybir.dt.bfloat16
    MULT = mybir.AluOpType.mult
    ADD = mybir.AluOpType.add

    class AF:
        Silu = (mybir.ActivationFunctionType.Sigmoid if _SIM_SAFE_ACT
                else mybir.ActivationFunctionType.Silu)
        Tanh = mybir.ActivationFunctionType.Tanh
        Identity = mybir.ActivationFunctionType.Identity

    nc = bacc.Bacc(trn_type="TRN2", target_bir_lowering=False, debug=False,
                   num_devices=N_CORES)

    # DRAM inputs (per-core shard) -----------------------------------------
    d_bigin = nc.dram_tensor("bigin", [128, T_TILES * BIGIN_W], BF16,
                             kind="ExternalInput")
    d_x1v2d = nc.dram_tensor("x1v2d", [128, E_PAD], BF16,
                             kind="ExternalInput")
    d_repsv2 = nc.dram_tensor("repsv2", [128, E_PAD], BF16,
                              kind="ExternalInput")
    # folded weights ([K, M] layouts, ready as lhsT)
    d_wa0 = nc.dram_tensor("wa0", [128, 128], BF16, kind="ExternalInput")
    d_wa1d = nc.dram_tensor("wa1d", [128, 128], BF16, kind="ExternalInput")
    d_wb4s = nc.dram_tensor("wb4s", [128, 128], BF16, kind="ExternalInput")
    d_wb5sd = nc.dram_tensor("wb5sd", [128, 128], BF16, kind="ExternalInput")
    d_wb4b = nc.dram_tensor("wb4b", [64, 128], BF16, kind="ExternalInput")
    d_wb5bd = nc.dram_tensor("wb5bd", [64, 128], BF16, kind="ExternalInput")
    d_wp2d = nc.dram_tensor("wp2d", [128, 128], BF16, kind="ExternalInput")
    d_wcd = nc.dram_tensor("wcd", [128, 128], BF16, kind="ExternalInput")
    d_wc = nc.dram_tensor("wc", [64, 64], BF16, kind="ExternalInput")
    d_fc0 = nc.dram_tensor("fc0", [128, 64], BF16, kind="ExternalInput")
    d_fc1 = nc.dram_tensor("fc1", [64, 64], BF16, kind="ExternalInput")
    d_fc2a = nc.dram_tensor("fc2a", [64, 128], BF16, kind="ExternalInput")
    d_fc2bd = nc.dram_tensor("fc2bd", [64, 128], BF16, kind="ExternalInput")
    d_b0 = nc.dram_tensor("b0c", [64, 1], F32, kind="ExternalInput")
    d_b1 = nc.dram_tensor("b1c", [64, 1], F32, kind="ExternalInput")
    d_b2a = nc.dram_tensor("b2a", [128, 1], F32, kind="ExternalInput")
    d_b2bd = nc.dram_tensor("b2bd", [128, 1], F32, kind="ExternalInput")

    d_bigout = nc.dram_tensor("bigout", [128, T_TILES * BIGOUT_W], BF16,
                              kind="ExternalOutput")
    d_out2 = nc.dram_tensor("out2", [64, E_PAD], BF16, kind="ExternalOutput")

    with tile.TileContext(nc) as tc:
        with tc.tile_pool(name="const", bufs=1) as cp, \
             tc.tile_pool(name="io", bufs=3) as io, \
             tc.tile_pool(name="wk", bufs=3) as wk, \
             tc.tile_pool(name="ot", bufs=3) as ot, \
             tc.tile_pool(name="ps", bufs=1, space="PSUM") as ps:

            def const(d, shape, dtype=BF16, lo=0):
                t = cp.tile(shape, dtype, name=d.name + "_sb")
                if lo:
                    nc.sync.dma_start(t[lo:128, :], d.ap())
                else:
                    nc.sync.dma_start(t, d.ap())
                return t

            w_wa0 = const(d_wa0, [128, 128])
            w_wa1d = const(d_wa1d, [128, 128])
            w_wb4s = const(d_wb4s, [128, 128])
            w_wb5sd = const(d_wb5sd, [128, 128])
            w_wp2d = const(d_wp2d, [128, 128])
            w_wcd = const(d_wcd, [128, 128])
            w_fc0 = const(d_fc0, [128, 64])
            w_wc = const(d_wc, [64, 64])
            w_fc1 = const(d_fc1, [64, 64])
            # weights living at partitions 64-127 (rhs at partition offset 64)
            w_wb4b = const(d_wb4b, [128, 128], lo=64)
            w_wb5bd = const(d_wb5bd, [128, 128], lo=64)
            w_fc2a = const(d_fc2a, [128, 128], lo=64)
            w_fc2bd = const(d_fc2bd, [128, 128], lo=64)
            c_b0 = const(d_b0, [64, 1], F32)
            c_b1 = const(d_b1, [128, 1], F32, lo=64)
            c_b2a = const(d_b2a, [128, 1], F32)
            c_b2bd = const(d_b2bd, [128, 1], F32)

            # Warm up the seed-accumulated PSUM banks: T8/T9 accumulate
            # with start=False onto DVE-seeded banks; the very first
            # group on a virgin bank misbehaves unless the bank has
            # hosted a started accumulation group once.
            zwarm = cp.tile([128, NH], BF16, name="zwarm")
            nc.vector.memset(zwarm, 0)
            for tag in ("c01", "c2"):
                pw = ps.tile([128, NH], F32, tag=tag)
                nc.tensor.matmul(pw, w_wa0, zwarm, start=True, stop=True)

            # software-pipelined emission: the wwa/wwb matmuls, their
            # evacuations, and the DVE tails + stores for half X are all
            # drained at the start of half X+1, so the PE never waits on
            # the in-half silu chain and every engine queue keeps the
            # seed/tail hazard alternation (no in-order deadlocks).
            pend = {}

            def drain_pend():
                if not pend:
                    return
                c = dict(pend)
                pend.clear()
                hs = c["hs"]
                nc.tensor.matmul(c["wwa"], w_fc2a[64:128, :],
                                 c["h2s"][64:128, hs],
                                 start=True, stop=True,
                                 tile_position=(64, 0))              # M3
                nc.tensor.matmul(c["wwb2"], w_fc2bd[64:128, :],
                                 c["h2s"][64:128, hs],
                                 start=True, stop=True,
                                 tile_position=(64, 0))              # M4
                nc.scalar.activation(c["wwa_sb"][:, hs], c["wwa"],
                                     AF.Identity, bias=c_b2a)        # A6
                nc.scalar.activation(c["wwb_sb"][:, hs], c["wwb2"],
                                     AF.Identity, bias=c_b2bd,
                                     scale=0.5)                      # A5
                # sgw2 = (tanh(g/2)+1) * (0.5*wwb2+0.5*b2b) = sig(g)*w_b
                nc.vector.scalar_tensor_tensor(c["sgw2"][:, hs],
                                               c["tg"][:, hs], 1.0,
                                               c["wwb_sb"][:, hs],
                                               ADD, MULT)
                # out_s = silu(scal) * (wwa + b2a)
                nc.vector.tensor_tensor(
                    c["bigout"][:, c["h"] * NH:(c["h"] + 1) * NH],
                    c["sc_silu"][:, hs], c["wwa_sb"][:, hs], MULT)
                nc.vector.tensor_tensor(
                    c["bigout"][:, NT + c["h"] * NH:NT + (c["h"] + 1) * NH],
                    c["c01"], c["sgw2"][:, hs], MULT)
                nc.vector.tensor_tensor(
                    c["out2"][64:128, hs], c["c2"][64:128, :],
                    c["sgw2"][64:128, hs], MULT)
                if c["lasth"]:
                    t = c["t"]
                    w = c["w"]
                    if w == NT:
                        nc.sync.dma_start(
                            d_bigout.ap()[:, t * BIGOUT_W:
                                          (t + 1) * BIGOUT_W],
                            c["bigout"])
                        nc.sync.dma_start(
                            d_out2.ap()[:, t * NT:(t + 1) * NT],
                            c["out2"][64:128, :])
                    else:
                        # short final supertile: store only valid columns
                        nc.sync.dma_start(
                            d_bigout.ap()[:, t * BIGOUT_W:t * BIGOUT_W + w],
                            c["bigout"][:, 0:w])
                        nc.sync.dma_start(
                            d_bigout.ap()[:, t * BIGOUT_W + NT:
                                          t * BIGOUT_W + NT + w],
                            c["bigout"][:, NT:NT + w])
                        nc.sync.dma_start(
                            d_out2.ap()[:, t * NT:t * NT + w],
                            c["out2"][64:128, 0:w])

            for t in range(T_TILES):
                # the last supertile only has 424 real edges: compute a
                # single 512-col half there instead of two
                nhalves = 1 if t == T_TILES - 1 else 2
                w = nhalves * NH
                sl = slice(t * NT, (t + 1) * NT)

                # ---- loads -------------------------------------------
                bigin = io.tile([128, BIGIN_W], BF16)
                nc.sync.dma_start(
                    bigin, d_bigin.ap()[:, t * BIGIN_W:(t + 1) * BIGIN_W])
                x1v2d = io.tile([128, NT], BF16)
                nc.sync.dma_start(x1v2d, d_x1v2d.ap()[:, sl])
                repsv2 = io.tile([128, NT], BF16)
                nc.sync.dma_start(repsv2, d_repsv2.ap()[:, sl])

                x1s = bigin[:, 0 * NT:1 * NT]
                x1v01 = bigin[:, 1 * NT:2 * NT]
                fw = bigin[:, 2 * NT:3 * NT]
                rep_s = bigin[:, 3 * NT:4 * NT]
                rep_v01 = bigin[:, 4 * NT:5 * NT]

                # ---- prescales: Pool (sbuf bf16), PE-consumption order.
                # First supertiles go on the (then idle) DVE so the PE
                # isn't gated on the slow Pool engine at startup.
                eng1 = nc.vector if t == 0 else nc.gpsimd
                eng2 = nc.vector if t <= 1 else nc.gpsimd
                # xvps2 = [x1v2*s (lo) ; x1v2*v2 (hi)]
                xvps2 = wk.tile([128, NT], BF16)
                eng2.tensor_tensor(xvps2[:, :w], x1v2d[:, :w],
                                   repsv2[:, :w], MULT)
                xs_s = wk.tile([128, NT], BF16)
                eng2.tensor_tensor(xs_s[:, :w], x1s[:, :w],
                                   rep_s[:, :w], MULT)
                xv_p01 = wk.tile([128, NT], BF16)
                eng1.tensor_tensor(xv_p01[:, :w], x1v01[:, :w],
                                   rep_v01[:, :w], MULT)
                xv_s01 = wk.tile([128, NT], BF16)
                eng1.tensor_tensor(xv_s01[:, :w], x1v01[:, :w],
                                   rep_s[:, :w], MULT)

                # ---- SBUF result tiles -------------------------------
                h1s = wk.tile([64, NT], BF16)
                h2s = wk.tile([128, NT], BF16)
                sc_silu = wk.tile([128, NT], BF16)
                tg = wk.tile([128, NT], BF16)
                wwa_sb = wk.tile([128, NT], BF16)
                wwb_sb = wk.tile([128, NT], BF16)
                sgw2 = wk.tile([128, NT], BF16)
                bigout = ot.tile([128, BIGOUT_W], BF16)
                out2 = ot.tile([128, NT], BF16)

                for h in range(nhalves):
                    hs = slice(h * NH, (h + 1) * NH)

                    a2 = ps.tile([128, NH], F32, tag="a2")
                    scal = ps.tile([128, NH], F32, tag="scal")
                    gate2 = ps.tile([128, NH], F32, tag="gate2")
                    mlp = ps.tile([128, NH], F32, tag="mlp")
                    c01 = ps.tile([128, NH], F32, tag="c01")
                    c2 = ps.tile([128, NH], F32, tag="c2")
                    wwa = ps.tile([128, NH], F32, tag="wwa")
                    wwb2 = ps.tile([128, NH], F32, tag="wwb2")

                    # ---- PE stream (readiness order) -----------------
                    nc.tensor.matmul(a2, w_wp2d, x1s[:, hs],
                                     start=True, stop=True)          # T7
                    nc.tensor.matmul(mlp[0:64, :], w_fc0, fw[:, hs],
                                     start=True, stop=True)          # M1
                    drain_pend()          # prev half: M3/M4, A6/A5, tails
                    nc.tensor.matmul(scal, w_wb4b[64:128, :],
                                     xvps2[64:128, hs],
                                     start=True, stop=False,
                                     tile_position=(64, 0))          # T3
                    nc.tensor.matmul(gate2, w_wb5bd[64:128, :],
                                     xvps2[64:128, hs],
                                     start=True, stop=False,
                                     tile_position=(64, 0))          # T6
                    nc.tensor.matmul(scal, w_wa0, xs_s[:, hs],
                                     start=False, stop=False)        # T1
                    nc.tensor.matmul(gate2, w_wa1d, xs_s[:, hs],
                                     start=False, stop=False)        # T4
                    nc.tensor.matmul(scal, w_wb4s, xv_p01[:, hs],
                                     start=False, stop=True)         # T2
                    nc.tensor.matmul(gate2, w_wb5sd, xv_p01[:, hs],
                                     start=False, stop=True)         # T5

                    # seed vec banks with the p2 postscale, then let the
                    # wc matmuls accumulate on top (start=False)
                    nc.vector.tensor_tensor(c01, a2, rep_v01[:, hs], MULT)
                    nc.vector.tensor_tensor(c2[64:128, :], a2[64:128, :],
                                            repsv2[64:128, hs], MULT)

                    # ---- radial MLP (wwa/wwb deferred to next half) --
                    nc.scalar.activation(h1s[:, hs], mlp[0:64, :], AF.Silu,
                                         bias=c_b0)                  # A1
                    nc.tensor.matmul(mlp[64:128, :], w_fc1, h1s[:, hs],
                                     start=True, stop=True,
                                     tile_position=(0, 64))          # M2
                    nc.tensor.matmul(c01, w_wcd, xv_s01[:, hs],
                                     start=False, stop=True,
                                     skip_group_check=True)          # T8
                    nc.tensor.matmul(c2[64:128, :], w_wc,
                                     xvps2[0:64, hs],
                                     start=False, stop=True,
                                     skip_group_check=True,
                                     tile_position=(0, 64))          # T9

                    nc.scalar.activation(sc_silu[:, hs], scal, AF.Silu)
                    nc.scalar.activation(tg[:, hs], gate2, AF.Tanh,
                                         scale=0.5)
                    nc.scalar.activation(h2s[64:128, hs], mlp[64:128, :],
                                         AF.Silu,
                                         bias=c_b1[64:128, :])       # A2
                    pend.update(t=t, h=h, hs=hs, h2s=h2s, wwa=wwa,
                                wwb2=wwb2, c01=c01, c2=c2,
                                sc_silu=sc_silu, tg=tg, wwa_sb=wwa_sb,
                                wwb_sb=wwb_sb, sgw2=sgw2, bigout=bigout,
                                out2=out2, lasth=(h == nhalves - 1), w=w)

            drain_pend()

    nc.compile()
    _CACHE["nc"] = nc
    return nc


def _bf16(x):
    import ml_dtypes
    return np.asarray(x, dtype=np.float32).astype(ml_dtypes.bfloat16)


def _fold_weights(inp):
    """Fold per-channel weights + constants into matmul matrices (bf16)."""
    f = lambda k: np.asarray(inp[k], dtype=np.float32)
    w0f = f("w1_p0") * f("w2_p0")[None, :] * (INV_S * SQ2)
    w1f = f("w1_p1") * f("w2_p1")[None, :] * (INV_S * SQ2)
    w2f = f("w1_p2") * f("w2_p2")[None, :] * (INV_S * SQ2)
    w3f = f("w1_p3") * f("w2_p3")[None, :] * (INV_V * SQ2)
    w4f = f("w1_p4") * f("w2_p4")[None, :] * (INV_V * SQ3 * SQ2)
    w5f = f("w1_p5") * f("w2_p5")[None, :] * (INV_V * SQ3 * SQ2)
    fc2 = f("fc_w2")
    b2 = f("fc_b2")
    wcd = np.zeros((128, 128), np.float32)
    wcd[:64, :64] = w3f
    wcd[64:, 64:] = w3f
    c = np.ascontiguousarray
    return {
        "wa0": _bf16(w0f),
        "wa1d": _bf16(np.concatenate([w1f, w1f], axis=1)),
        "wb4s": _bf16(np.concatenate([w4f, w4f], axis=0)),
        "wb5sd": _bf16(np.tile(w5f, (2, 2))),
        "wb4b": _bf16(w4f),
        "wb5bd": _bf16(np.concatenate([w5f, w5f], axis=1)),
        "wp2d": _bf16(np.concatenate([w2f, w2f], axis=1)),
        "wcd": _bf16(wcd),
        "wc": _bf16(w3f),
        "fc0": _bf16(f("fc_w0")),
        "fc1": _bf16(f("fc_w1")),
        "fc2a": _bf16(fc2[:, :128]),
        "fc2bd": _bf16(np.concatenate([fc2[:, 128:], fc2[:, 128:]], axis=1)),
        "b0c": c(f("fc_b0")[:, None]),
        "b1c": c(f("fc_b1")[:, None]),
        "b2a": c(b2[:128, None]),
        "b2bd": c(0.5 * np.concatenate([b2[128:], b2[128:]])[:, None]),
    }


def _shard_inputs(inp):
    """Per-core bf16 channel-major shards (padded to E_PAD edges)."""
    import ml_dtypes
    BF = ml_dtypes.bfloat16
    fea_in1 = np.asarray(inp["fea_in1"], dtype=np.float32)
    fea_in2 = np.asarray(inp["fea_in2"], dtype=np.float32)
    fea_w = np.asarray(inp["fea_weight"], dtype=np.float32)
    shards = []
    for c in range(N_CORES):
        s = slice(c * E_CORE, (c + 1) * E_CORE)
        x1 = fea_in1[s]
        x2 = fea_in2[s]
        fwm = fea_w[s]

        x1s_t = np.zeros((128, E_PAD), BF)
        x1s_t[:, :E_CORE] = x1[:, :128].T
        x1v = x1[:, 128:].reshape(E_CORE, 64, 3)
        x1v01_t = np.zeros((128, E_PAD), BF)
        x1v01_t[:64, :E_CORE] = x1v[:, :, 0].T
        x1v01_t[64:, :E_CORE] = x1v[:, :, 1].T
        x1v2d_t = np.zeros((128, E_PAD), BF)
        x1v2d_t[:64, :E_CORE] = x1v[:, :, 2].T
        x1v2d_t[64:, :E_CORE] = x1v[:, :, 2].T
        fw_t = np.zeros((128, E_PAD), BF)
        fw_t[:, :E_CORE] = fwm.T
        x2p = np.zeros((E_PAD, 4), np.float32)
        x2p[:E_CORE] = x2

        rep_s = np.broadcast_to(
            x2p[:, 0].astype(BF)[None, :], (128, E_PAD))
        rep_v01 = np.empty((128, E_PAD), BF)
        rep_v01[:64] = x2p[:, 1].astype(BF)
        rep_v01[64:] = x2p[:, 2].astype(BF)
        repsv2 = np.empty((128, E_PAD), BF)
        repsv2[:64] = x2p[:, 0].astype(BF)
        repsv2[64:] = x2p[:, 3].astype(BF)

        # interleave into bigin: [128, T, 5, NT]
        big = np.stack([
            x1s_t.reshape(128, T_TILES, NT),
            x1v01_t.reshape(128, T_TILES, NT),
            fw_t.reshape(128, T_TILES, NT),
            np.ascontiguousarray(rep_s).reshape(128, T_TILES, NT),
            rep_v01.reshape(128, T_TILES, NT),
        ], axis=2)                                  # [128, T, 5, NT]
        shards.append({
            "bigin": np.ascontiguousarray(
                big.reshape(128, T_TILES * BIGIN_W)),
            "x1v2d": x1v2d_t,
            "repsv2": repsv2,
        })
    return shards


def run(inputs, trace=False, trace_kwargs=None):
    """Run the kernel; returns (output [E,320] f32, BassKernelResults)."""
    _ensure_repo_on_path()
    from concourse import bass_utils

    nc = _build_nc()
    weights = _fold_weights(inputs)
    shards = _shard_inputs(inputs)
    in_maps = [{**weights, **sh} for sh in shards]

    kwargs = {}
    if trace:
        _install_ntff_hook()
        kwargs.update(trace=True, **(trace_kwargs or {}))
    res = bass_utils.run_bass_kernel_spmd(
        nc, in_maps, core_ids=list(range(N_CORES)), **kwargs)

    out = np.empty((E_FULL, 320), np.float32)
    for c in range(N_CORES):
        bo = np.asarray(res.results[c]["bigout"], dtype=np.float32)
        o2 = np.asarray(res.results[c]["out2"], dtype=np.float32)
        bo = bo.reshape(128, T_TILES, 2, NT)
        out_s = bo[:, :, 0, :].reshape(128, E_PAD)[:, :E_CORE]
        out01 = bo[:, :, 1, :].reshape(128, E_PAD)[:, :E_CORE]
        s = slice(c * E_CORE, (c + 1) * E_CORE)
        out[s, :128] = out_s.T
        # vec layout: out[e, 128 + u*3 + i]
        vec = np.empty((E_CORE, 64, 3), np.float32)
        vec[:, :, 0] = out01[:64].T
        vec[:, :, 1] = out01[64:].T
        vec[:, :, 2] = o2[:, :E_CORE].T
        out[s, 128:] = vec.reshape(E_CORE, 192)
    return out, res


def _install_ntff_hook():
    """Shim the missing antenv.axon_hooks so trace=True works under axon."""
    import types
    import antenv
    from concourse import bass_utils
    if "antenv.axon_hooks" in sys.modules:
        return
    mod = types.ModuleType("antenv.axon_hooks")
    _h = [None]
    mod.set_axon_ntff_profile_hook = lambda h: _h.__setitem__(0, h)
    mod.get_axon_ntff_profile_hook = lambda: _h[0]
    sys.modules["antenv.axon_hooks"] = mod
    antenv.axon_hooks = mod
    from trn_agent_boot.trn_boot import _ntff_profile_via_ctypes
    mod.set_axon_ntff_profile_hook(
        _ntff_profile_via_ctypes("/opt/axon/libaxon_pjrt.so"))
    bass_utils.upload_artifacts = lambda tmpdir: tmpdir


def kernel(**inputs) -> np.ndarray:
    out, _ = run(inputs, trace=False)
    return out


# revision 29
# speedup vs baseline: 1.0576x; 1.0173x over previous
"""EquiConv (DeepH-E3) Trainium2 kernel — 8-core data-parallel over edges.

Strategy (channel-major, bf16, 13 matmul slots per 1024-edge supertile):
  - Host folds per-channel weights/constants into matmul weights, casts
    everything to bf16, shards edges across 8 cores, pads to 25*1024 and
    transposes edge tensors to channel-major [C, E].  Per-edge scalars
    (s, v0, v1, v2 from fea_in2) are shipped pre-replicated across
    partitions in bf16; x1s / x1v01 / fw / rep_s / rep_v01 are packed
    into ONE interleaved "bigin" DRAM tensor so each supertile needs
    only 3 load DMAs + 2 store DMAs.
  - Matmul packing: gate and the p2 output are duplicated across both
    PSUM halves via widened weight matrices (idle M side), p3's wc
    matmuls for components 0/1 are merged into one block-diagonal
    matmul, and the vec-path adds are folded into PSUM accumulation by
    seeding the bank with the DVE product (matmul start=False).
  - Elementwise split: GpSimd(Pool) takes the three big SBUF-only
    prescales, ACT takes activations + the wwa/wwb PSUM evacuations,
    DVE takes the PSUM-touching products and bf16 finals (2x mode).
  - Host transposes the bf16 channel-major output back to [E, 320] f32.

Self-contained: hardcodes shapes from the problem spec; no file reads.
"""
import os
import sys

import numpy as np

# ---------------------------------------------------------------- constants
E_FULL = 200000
N_CORES = 8
E_CORE = E_FULL // N_CORES      # 25000
NT = 1024                       # edges per supertile
T_TILES = 25                    # supertiles per core
E_PAD = NT * T_TILES            # 25600
NH = 512                        # psum half
MUL_S = 128
MUL_V = 64

INV_S = 1.0 / np.sqrt(MUL_S)
INV_V = 1.0 / np.sqrt(MUL_V)
SQ2 = 1.0 / np.sqrt(2.0)
SQ3 = 1.0 / np.sqrt(3.0)

BIGIN_W = 5 * NT                # x1s | x1v01 | fw | rep_s | rep_v01
BIGOUT_W = 2 * NT               # out_s | out01

_REPO_CANDIDATES = (
    "/opt/trn_rl_repo",
    "/root/.axon_site/_ro/trn_rl_repo",
)


def _ensure_repo_on_path():
    try:
        import concourse.bass  # noqa: F401
        return
    except ImportError:
        pass
    for p in _REPO_CANDIDATES:
        if os.path.isdir(p) and p not in sys.path:
            sys.path.insert(0, p)
    import concourse.bass  # noqa: F401


_CACHE = {}
_SIM_SAFE_ACT = False   # replace Silu->Sigmoid so CoreSim can run


def _build_nc():
    """Build + compile the per-core Bass program (cached)."""
    if "nc" in _CACHE:
        return _CACHE["nc"]
    _ensure_repo_on_path()
    import concourse.mybir as mybir
    import concourse.tile as tile
    from concourse import bacc

    F32 = mybir.dt.float32
    BF16 = mybir.dt.bfloat16
    MULT = mybir.AluOpType.mult
    ADD = mybir.AluOpType.add

    class AF:
        Silu = (mybir.ActivationFunctionType.Sigmoid if _SIM_SAFE_ACT
                else mybir.ActivationFunctionType.Silu)
        Tanh = mybir.ActivationFunctionType.Tanh
        Identity = mybir.ActivationFunctionType.Identity

    nc = bacc.Bacc(trn_type="TRN2", target_bir_lowering=False, debug=False,
                   num_devices=N_CORES)

    # DRAM inputs (per-core shard) -----------------------------------------
    d_bigin = nc.dram_tensor("bigin", [128, T_TILES * BIGIN_W], BF16,
                             kind="ExternalInput")
    d_x1v2d = nc.dram_tensor("x1v2d", [128, E_PAD], BF16,
                             kind="ExternalInput")
    d_repsv2 = nc.dram_tensor("repsv2", [128, E_PAD], BF16,
                              kind="ExternalInput")
    # folded weights ([K, M] layouts, ready as lhsT)
    d_wa0 = nc.dram_tensor("wa0", [128, 128], BF16, kind="ExternalInput")
    d_wa1d = nc.dram_tensor("wa1d", [128, 128], BF16, kind="ExternalInput")
    d_wb4s = nc.dram_tensor("wb4s", [128, 128], BF16, kind="ExternalInput")
    d_wb5sd = nc.dram_tensor("wb5sd", [128, 128], BF16, kind="ExternalInput")
    d_wb4b = nc.dram_tensor("wb4b", [64, 128], BF16, kind="ExternalInput")
    d_wb5bd = nc.dram_tensor("wb5bd", [64, 128], BF16, kind="ExternalInput")
    d_wp2d = nc.dram_tensor("wp2d", [128, 128], BF16, kind="ExternalInput")
    d_wcd = nc.dram_tensor("wcd", [128, 128], BF16, kind="ExternalInput")
    d_wc = nc.dram_tensor("wc", [64, 64], BF16, kind="ExternalInput")
    d_fc0 = nc.dram_tensor("fc0", [128, 64], BF16, kind="ExternalInput")
    d_fc1 = nc.dram_tensor("fc1", [64, 64], BF16, kind="ExternalInput")
    d_fc2a = nc.dram_tensor("fc2a", [64, 128], BF16, kind="ExternalInput")
    d_fc2bd = nc.dram_tensor("fc2bd", [64, 128], BF16, kind="ExternalInput")
    d_b0 = nc.dram_tensor("b0c", [64, 1], F32, kind="ExternalInput")
    d_b1 = nc.dram_tensor("b1c", [64, 1], F32, kind="ExternalInput")
    d_b2a = nc.dram_tensor("b2a", [128, 1], F32, kind="ExternalInput")
    d_b2bd = nc.dram_tensor("b2bd", [128, 1], F32, kind="ExternalInput")

    d_bigout = nc.dram_tensor("bigout", [128, T_TILES * BIGOUT_W], BF16,
                              kind="ExternalOutput")
    d_out2 = nc.dram_tensor("out2", [64, E_PAD], BF16, kind="ExternalOutput")

    with tile.TileContext(nc) as tc:
        with tc.tile_pool(name="const", bufs=1) as cp, \
             tc.tile_pool(name="io", bufs=3) as io, \
             tc.tile_pool(name="wk", bufs=3) as wk, \
             tc.tile_pool(name="ot", bufs=3) as ot, \
             tc.tile_pool(name="ps", bufs=1, space="PSUM") as ps:

            def const(d, shape, dtype=BF16, lo=0):
                # issue const loads from the (startup-idle) Scalar queue
                # so the first data loads aren't stuck behind 17 issue
                # slots (~600ns each) on the Sync queue
                t = cp.tile(shape, dtype, name=d.name + "_sb")
                if lo:
                    nc.scalar.dma_start(t[lo:128, :], d.ap())
                else:
                    nc.scalar.dma_start(t, d.ap())
                return t

            w_wa0 = const(d_wa0, [128, 128])
            w_wa1d = const(d_wa1d, [128, 128])
            w_wb4s = const(d_wb4s, [128, 128])
            w_wb5sd = const(d_wb5sd, [128, 128])
            w_wp2d = const(d_wp2d, [128, 128])
            w_wcd = const(d_wcd, [128, 128])
            w_fc0 = const(d_fc0, [128, 64])
            w_wc = const(d_wc, [64, 64])
            w_fc1 = const(d_fc1, [64, 64])
            # weights living at partitions 64-127 (rhs at partition offset 64)
            w_wb4b = const(d_wb4b, [128, 128], lo=64)
            w_wb5bd = const(d_wb5bd, [128, 128], lo=64)
            w_fc2a = const(d_fc2a, [128, 128], lo=64)
            w_fc2bd = const(d_fc2bd, [128, 128], lo=64)
            c_b0 = const(d_b0, [64, 1], F32)
            c_b1 = const(d_b1, [128, 1], F32, lo=64)
            c_b2a = const(d_b2a, [128, 1], F32)
            c_b2bd = const(d_b2bd, [128, 1], F32)

            # Warm up the seed-accumulated PSUM banks: T8/T9 accumulate
            # with start=False onto DVE-seeded banks; the very first
            # group on a virgin bank misbehaves unless the bank has
            # hosted a started accumulation group once.
            zwarm = cp.tile([128, NH], BF16, name="zwarm")
            nc.vector.memset(zwarm, 0)
            for tag in ("c01", "c2"):
                pw = ps.tile([128, NH], F32, tag=tag)
                nc.tensor.matmul(pw, w_wa0, zwarm, start=True, stop=True)

            # software-pipelined emission: the wwa/wwb matmuls, their
            # evacuations, and the DVE tails + stores for half X are all
            # drained at the start of half X+1, so the PE never waits on
            # the in-half silu chain and every engine queue keeps the
            # seed/tail hazard alternation (no in-order deadlocks).
            pend = {}

            def drain_pend():
                if not pend:
                    return
                c = dict(pend)
                pend.clear()
                hs = c["hs"]
                nc.tensor.matmul(c["wwa"], w_fc2a[64:128, :],
                                 c["h2s"][64:128, hs],
                                 start=True, stop=True,
                                 tile_position=(64, 0))              # M3
                nc.tensor.matmul(c["wwb2"], w_fc2bd[64:128, :],
                                 c["h2s"][64:128, hs],
                                 start=True, stop=True,
                                 tile_position=(64, 0))              # M4
                nc.scalar.activation(c["wwa_sb"][:, hs], c["wwa"],
                                     AF.Identity, bias=c_b2a)        # A6
                nc.scalar.activation(c["wwb_sb"][:, hs], c["wwb2"],
                                     AF.Identity, bias=c_b2bd,
                                     scale=0.5)                      # A5
                # sgw2 = (tanh(g/2)+1) * (0.5*wwb2+0.5*b2b) = sig(g)*w_b
                nc.vector.scalar_tensor_tensor(c["sgw2"][:, hs],
                                               c["tg"][:, hs], 1.0,
                                               c["wwb_sb"][:, hs],
                                               ADD, MULT)
                # out_s = silu(scal) * (wwa + b2a)
                nc.vector.tensor_tensor(
                    c["bigout"][:, c["h"] * NH:(c["h"] + 1) * NH],
                    c["sc_silu"][:, hs], c["wwa_sb"][:, hs], MULT)
                nc.vector.tensor_tensor(
                    c["bigout"][:, NT + c["h"] * NH:NT + (c["h"] + 1) * NH],
                    c["c01"], c["sgw2"][:, hs], MULT)
                nc.vector.tensor_tensor(
                    c["out2"][64:128, hs], c["c2"][64:128, :],
                    c["sgw2"][64:128, hs], MULT)
                if c["lasth"]:
                    t = c["t"]
                    w = c["w"]
                    if w == NT:
                        nc.sync.dma_start(
                            d_bigout.ap()[:, t * BIGOUT_W:
                                          (t + 1) * BIGOUT_W],
                            c["bigout"])
                        nc.sync.dma_start(
                            d_out2.ap()[:, t * NT:(t + 1) * NT],
                            c["out2"][64:128, :])
                    else:
                        # short final supertile: store only valid columns
                        nc.sync.dma_start(
                            d_bigout.ap()[:, t * BIGOUT_W:t * BIGOUT_W + w],
                            c["bigout"][:, 0:w])
                        nc.sync.dma_start(
                            d_bigout.ap()[:, t * BIGOUT_W + NT:
                                          t * BIGOUT_W + NT + w],
                            c["bigout"][:, NT:NT + w])
                        nc.sync.dma_start(
                            d_out2.ap()[:, t * NT:t * NT + w],
                            c["out2"][64:128, 0:w])

            for t in range(T_TILES):
                # the last supertile only has 424 real edges: compute a
                # single 512-col half there instead of two
                nhalves = 1 if t == T_TILES - 1 else 2
                w = nhalves * NH
                sl = slice(t * NT, (t + 1) * NT)

                # ---- loads -------------------------------------------
                bigin = io.tile([128, BIGIN_W], BF16)
                nc.sync.dma_start(
                    bigin, d_bigin.ap()[:, t * BIGIN_W:(t + 1) * BIGIN_W])
                x1v2d = io.tile([128, NT], BF16)
                nc.sync.dma_start(x1v2d, d_x1v2d.ap()[:, sl])
                repsv2 = io.tile([128, NT], BF16)
                nc.sync.dma_start(repsv2, d_repsv2.ap()[:, sl])

                x1s = bigin[:, 0 * NT:1 * NT]
                x1v01 = bigin[:, 1 * NT:2 * NT]
                fw = bigin[:, 2 * NT:3 * NT]
                rep_s = bigin[:, 3 * NT:4 * NT]
                rep_v01 = bigin[:, 4 * NT:5 * NT]

                # ---- prescales: Pool (sbuf bf16), PE-consumption order.
                # First supertiles go on the (then idle) DVE so the PE
                # isn't gated on the slow Pool engine at startup.
                eng1 = nc.vector if t == 0 else nc.gpsimd
                eng2 = nc.vector if t <= 1 else nc.gpsimd
                # xvps2 = [x1v2*s (lo) ; x1v2*v2 (hi)]
                xvps2 = wk.tile([128, NT], BF16)
                eng2.tensor_tensor(xvps2[:, :w], x1v2d[:, :w],
                                   repsv2[:, :w], MULT)
                xs_s = wk.tile([128, NT], BF16)
                eng2.tensor_tensor(xs_s[:, :w], x1s[:, :w],
                                   rep_s[:, :w], MULT)
                xv_p01 = wk.tile([128, NT], BF16)
                eng1.tensor_tensor(xv_p01[:, :w], x1v01[:, :w],
                                   rep_v01[:, :w], MULT)
                xv_s01 = wk.tile([128, NT], BF16)
                eng1.tensor_tensor(xv_s01[:, :w], x1v01[:, :w],
                                   rep_s[:, :w], MULT)

                # ---- SBUF result tiles -------------------------------
                h1s = wk.tile([64, NT], BF16)
                h2s = wk.tile([128, NT], BF16)
                sc_silu = wk.tile([128, NT], BF16)
                tg = wk.tile([128, NT], BF16)
                wwa_sb = wk.tile([128, NT], BF16)
                wwb_sb = wk.tile([128, NT], BF16)
                sgw2 = wk.tile([128, NT], BF16)
                bigout = ot.tile([128, BIGOUT_W], BF16)
                out2 = ot.tile([128, NT], BF16)

                for h in range(nhalves):
                    hs = slice(h * NH, (h + 1) * NH)

                    a2 = ps.tile([128, NH], F32, tag="a2")
                    scal = ps.tile([128, NH], F32, tag="scal")
                    gate2 = ps.tile([128, NH], F32, tag="gate2")
                    mlp = ps.tile([128, NH], F32, tag="mlp")
                    c01 = ps.tile([128, NH], F32, tag="c01")
                    c2 = ps.tile([128, NH], F32, tag="c2")
                    wwa = ps.tile([128, NH], F32, tag="wwa")
                    wwb2 = ps.tile([128, NH], F32, tag="wwb2")

                    # ---- PE stream (readiness order) -----------------
                    nc.tensor.matmul(a2, w_wp2d, x1s[:, hs],
                                     start=True, stop=True)          # T7
                    nc.tensor.matmul(mlp[0:64, :], w_fc0, fw[:, hs],
                                     start=True, stop=True)          # M1
                    drain_pend()          # prev half: M3/M4, A6/A5, tails
                    nc.tensor.matmul(scal, w_wb4b[64:128, :],
                                     xvps2[64:128, hs],
                                     start=True, stop=False,
                                     tile_position=(64, 0))          # T3
                    nc.tensor.matmul(gate2, w_wb5bd[64:128, :],
                                     xvps2[64:128, hs],
                                     start=True, stop=False,
                                     tile_position=(64, 0))          # T6
                    nc.tensor.matmul(scal, w_wa0, xs_s[:, hs],
                                     start=False, stop=False)        # T1
                    nc.tensor.matmul(gate2, w_wa1d, xs_s[:, hs],
                                     start=False, stop=False)        # T4
                    nc.tensor.matmul(scal, w_wb4s, xv_p01[:, hs],
                                     start=False, stop=True)         # T2
                    nc.tensor.matmul(gate2, w_wb5sd, xv_p01[:, hs],
                                     start=False, stop=True)         # T5

                    # seed vec banks with the p2 postscale, then let the
                    # wc matmuls accumulate on top (start=False)
                    nc.vector.tensor_tensor(c01, a2, rep_v01[:, hs], MULT)
                    nc.vector.tensor_tensor(c2[64:128, :], a2[64:128, :],
                                            repsv2[64:128, hs], MULT)

                    # ---- radial MLP (wwa/wwb deferred to next half) --
                    nc.scalar.activation(h1s[:, hs], mlp[0:64, :], AF.Silu,
                                         bias=c_b0)                  # A1
                    nc.tensor.matmul(mlp[64:128, :], w_fc1, h1s[:, hs],
                                     start=True, stop=True,
                                     tile_position=(0, 64))          # M2
                    nc.tensor.matmul(c01, w_wcd, xv_s01[:, hs],
                                     start=False, stop=True,
                                     skip_group_check=True)          # T8
                    nc.tensor.matmul(c2[64:128, :], w_wc,
                                     xvps2[0:64, hs],
                                     start=False, stop=True,
                                     skip_group_check=True,
                                     tile_position=(0, 64))          # T9

                    nc.scalar.activation(sc_silu[:, hs], scal, AF.Silu)
                    nc.scalar.activation(tg[:, hs], gate2, AF.Tanh,
                                         scale=0.5)
                    nc.scalar.activation(h2s[64:128, hs], mlp[64:128, :],
                                         AF.Silu,
                                         bias=c_b1[64:128, :])       # A2
                    pend.update(t=t, h=h, hs=hs, h2s=h2s, wwa=wwa,
                                wwb2=wwb2, c01=c01, c2=c2,
                                sc_silu=sc_silu, tg=tg, wwa_sb=wwa_sb,
                                wwb_sb=wwb_sb, sgw2=sgw2, bigout=bigout,
                                out2=out2, lasth=(h == nhalves - 1), w=w)

            drain_pend()

    nc.compile()
    _CACHE["nc"] = nc
    return nc


def _bf16(x):
    import ml_dtypes
    return np.asarray(x, dtype=np.float32).astype(ml_dtypes.bfloat16)


def _fold_weights(inp):
    """Fold per-channel weights + constants into matmul matrices (bf16)."""
    f = lambda k: np.asarray(inp[k], dtype=np.float32)
    w0f = f("w1_p0") * f("w2_p0")[None, :] * (INV_S * SQ2)
    w1f = f("w1_p1") * f("w2_p1")[None, :] * (INV_S * SQ2)
    w2f = f("w1_p2") * f("w2_p2")[None, :] * (INV_S * SQ2)
    w3f = f("w1_p3") * f("w2_p3")[None, :] * (INV_V * SQ2)
    w4f = f("w1_p4") * f("w2_p4")[None, :] * (INV_V * SQ3 * SQ2)
    w5f = f("w1_p5") * f("w2_p5")[None, :] * (INV_V * SQ3 * SQ2)
    fc2 = f("fc_w2")
    b2 = f("fc_b2")
    wcd = np.zeros((128, 128), np.float32)
    wcd[:64, :64] = w3f
    wcd[64:, 64:] = w3f
    c = np.ascontiguousarray
    return {
        "wa0": _bf16(w0f),
        "wa1d": _bf16(np.concatenate([w1f, w1f], axis=1)),
        "wb4s": _bf16(np.concatenate([w4f, w4f], axis=0)),
        "wb5sd": _bf16(np.tile(w5f, (2, 2))),
        "wb4b": _bf16(w4f),
        "wb5bd": _bf16(np.concatenate([w5f, w5f], axis=1)),
        "wp2d": _bf16(np.concatenate([w2f, w2f], axis=1)),
        "wcd": _bf16(wcd),
        "wc": _bf16(w3f),
        "fc0": _bf16(f("fc_w0")),
        "fc1": _bf16(f("fc_w1")),
        "fc2a": _bf16(fc2[:, :128]),
        "fc2bd": _bf16(np.concatenate([fc2[:, 128:], fc2[:, 128:]], axis=1)),
        "b0c": c(f("fc_b0")[:, None]),
        "b1c": c(f("fc_b1")[:, None]),
        "b2a": c(b2[:128, None]),
        "b2bd": c(0.5 * np.concatenate([b2[128:], b2[128:]])[:, None]),
    }


def _shard_inputs(inp):
    """Per-core bf16 channel-major shards (padded to E_PAD edges)."""
    import ml_dtypes
    BF = ml_dtypes.bfloat16
    fea_in1 = np.asarray(inp["fea_in1"], dtype=np.float32)
    fea_in2 = np.asarray(inp["fea_in2"], dtype=np.float32)
    fea_w = np.asarray(inp["fea_weight"], dtype=np.float32)
    shards = []
    for c in range(N_CORES):
        s = slice(c * E_CORE, (c + 1) * E_CORE)
        x1 = fea_in1[s]
        x2 = fea_in2[s]
        fwm = fea_w[s]

        x1s_t = np.zeros((128, E_PAD), BF)
        x1s_t[:, :E_CORE] = x1[:, :128].T
        x1v = x1[:, 128:].reshape(E_CORE, 64, 3)
        x1v01_t = np.zeros((128, E_PAD), BF)
        x1v01_t[:64, :E_CORE] = x1v[:, :, 0].T
        x1v01_t[64:, :E_CORE] = x1v[:, :, 1].T
        x1v2d_t = np.zeros((128, E_PAD), BF)
        x1v2d_t[:64, :E_CORE] = x1v[:, :, 2].T
        x1v2d_t[64:, :E_CORE] = x1v[:, :, 2].T
        fw_t = np.zeros((128, E_PAD), BF)
        fw_t[:, :E_CORE] = fwm.T
        x2p = np.zeros((E_PAD, 4), np.float32)
        x2p[:E_CORE] = x2

        rep_s = np.broadcast_to(
            x2p[:, 0].astype(BF)[None, :], (128, E_PAD))
        rep_v01 = np.empty((128, E_PAD), BF)
        rep_v01[:64] = x2p[:, 1].astype(BF)
        rep_v01[64:] = x2p[:, 2].astype(BF)
        repsv2 = np.empty((128, E_PAD), BF)
        repsv2[:64] = x2p[:, 0].astype(BF)
        repsv2[64:] = x2p[:, 3].astype(BF)

        # interleave into bigin: [128, T, 5, NT]
        big = np.stack([
            x1s_t.reshape(128, T_TILES, NT),
            x1v01_t.reshape(128, T_TILES, NT),
            fw_t.reshape(128, T_TILES, NT),
            np.ascontiguousarray(rep_s).reshape(128, T_TILES, NT),
            rep_v01.reshape(128, T_TILES, NT),
        ], axis=2)                                  # [128, T, 5, NT]
        shards.append({
            "bigin": np.ascontiguousarray(
                big.reshape(128, T_TILES * BIGIN_W)),
            "x1v2d": x1v2d_t,
            "repsv2": repsv2,
        })
    return shards


def run(inputs, trace=False, trace_kwargs=None):
    """Run the kernel; returns (output [E,320] f32, BassKernelResults)."""
    _ensure_repo_on_path()
    from concourse import bass_utils

    nc = _build_nc()
    weights = _fold_weights(inputs)
    shards = _shard_inputs(inputs)
    in_maps = [{**weights, **sh} for sh in shards]

    kwargs = {}
    if trace:
        _install_ntff_hook()
        kwargs.update(trace=True, **(trace_kwargs or {}))
    res = bass_utils.run_bass_kernel_spmd(
        nc, in_maps, core_ids=list(range(N_CORES)), **kwargs)

    out = np.empty((E_FULL, 320), np.float32)
    for c in range(N_CORES):
        bo = np.asarray(res.results[c]["bigout"], dtype=np.float32)
        o2 = np.asarray(res.results[c]["out2"], dtype=np.float32)
        bo = bo.reshape(128, T_TILES, 2, NT)
        out_s = bo[:, :, 0, :].reshape(128, E_PAD)[:, :E_CORE]
        out01 = bo[:, :, 1, :].reshape(128, E_PAD)[:, :E_CORE]
        s = slice(c * E_CORE, (c + 1) * E_CORE)
        out[s, :128] = out_s.T
        # vec layout: out[e, 128 + u*3 + i]
        vec = np.empty((E_CORE, 64, 3), np.float32)
        vec[:, :, 0] = out01[:64].T
        vec[:, :, 1] = out01[64:].T
        vec[:, :, 2] = o2[:, :E_CORE].T
        out[s, 128:] = vec.reshape(E_CORE, 192)
    return out, res


def _install_ntff_hook():
    """Shim the missing antenv.axon_hooks so trace=True works under axon."""
    import types
    import antenv
    from concourse import bass_utils
    if "antenv.axon_hooks" in sys.modules:
        return
    mod = types.ModuleType("antenv.axon_hooks")
    _h = [None]
    mod.set_axon_ntff_profile_hook = lambda h: _h.__setitem__(0, h)
    mod.get_axon_ntff_profile_hook = lambda: _h[0]
    sys.modules["antenv.axon_hooks"] = mod
    antenv.axon_hooks = mod
    from trn_agent_boot.trn_boot import _ntff_profile_via_ctypes
    mod.set_axon_ntff_profile_hook(
        _ntff_profile_via_ctypes("/opt/axon/libaxon_pjrt.so"))
    bass_utils.upload_artifacts = lambda tmpdir: tmpdir


def kernel(**inputs) -> np.ndarray:
    out, _ = run(inputs, trace=False)
    return out
